# revision 2
# baseline (speedup 1.0000x reference)
"""Trainium2 Bass kernel for nn_AttnGlobal (B=8, N=4096, DIM=128).

reference:
    kv = x @ Wkv + bkv ; k, v = split(kv)
    q = q_global / sqrt(d)
    scores = einsum("bnd,bmd->bnm", k, q)       # softmax over m
    attn = softmax(scores, axis=-1)
    out = einsum("bnm,bmd->bnd", attn, v) @ Wp + bp

Sharding: pure data-parallel over B across the 8 cores (one batch each).

Host-side algebra folds:
    w   = x @ (Wv @ Wp)            (since attn @ (x@Wv) @ Wp = attn @ (x@(Wv@Wp)))
    bpe = bv @ Wp + bp             (since rows of attn sum to 1)

Per-core dataflow:
    xT, qT  : host-pretransposed fp16 inputs        [d, n] / [d, m]
    kT      = Wk.T @ xT + bk                        [d, n]   fp16
    S.T     = qT.T-chunks @ kT                      [m, n] tiles in PSUM (fp32)
    E.T     = exp(S.T / sqrt(d))                    fp16, ACT straight from PSUM
    U_aug   = E @ [w | 1]                           [n, 129] accumulated in PSUM
    out     = U[:, :128] * (1 / U[:, 128]) + bpe    DVE, then DMA out

Schedule: the ACT engine's exp stream (16.8M elems/core @ 1 elem/cyc/lane)
is the hard floor (~133us). The software pipeline keeps ACT gapless:
per chunk c we emit S-groups of chunk c+1 interleaved with U-batches of
chunk c (which consume exp output written one chunk-iteration earlier),
so the PE fills its ACT-slot-wait gaps with U matmuls instead of idling.
DMA triggers live on sync/gpsimd queues (never the scalar engine).
"""

import os
import sys

try:
    import concourse  # noqa: F401  (resolvable via PYTHONPATH on axon images)
except ImportError:
    for _p in ("/opt/trn_rl_repo", os.path.expanduser("~/.axon_site/_ro/trn_rl_repo")):
        if os.path.isdir(_p) and _p not in sys.path:
            sys.path.append(_p)

import numpy as np

import concourse.bacc as bacc
import concourse.mybir as mybir
from concourse.bass_utils import run_bass_kernel_spmd
from concourse.tile import TileContext

B, N, D = 8, 4096, 128
NT = N // 128          # 32 row tiles
NC = N // 512          # 8 column chunks
F32 = mybir.dt.float32
F16 = mybir.dt.float16
EXP_SCALE = 1.0 / float(np.sqrt(D))

# alternating PSUM score-group sizes; sum == NT, st4 uses 4 banks, st2 uses 2
S_GROUPS = [2, 4, 2, 4, 2, 4, 2, 4, 2, 4, 2]
assert sum(S_GROUPS) == NT
S_STARTS = [sum(S_GROUPS[:i]) for i in range(len(S_GROUPS))]


def build(reps: int = 1):
    """Build and compile the per-core Bass program (identical on all cores)."""
    nc = bacc.Bacc("TRN2", target_bir_lowering=False)

    xt = nc.dram_tensor("xt", [D, N], F16, kind="ExternalInput")
    qt = nc.dram_tensor("qt", [D, N], F16, kind="ExternalInput")
    wk = nc.dram_tensor("wk", [D, D], F16, kind="ExternalInput")
    wvp = nc.dram_tensor("wvp", [D, D], F16, kind="ExternalInput")
    bk = nc.dram_tensor("bk", [D, 1], F32, kind="ExternalInput")
    bpe = nc.dram_tensor("bpe", [D, D], F32, kind="ExternalInput")  # row-tiled bias
    out = nc.dram_tensor("out", [N, D], F32, kind="ExternalOutput")

    with TileContext(nc) as tc:
        xTc = [nc.alloc_sbuf_tensor(f"xT{c}", [128, 512], F16) for c in range(NC)]
        qTc = [nc.alloc_sbuf_tensor(f"qT{c}", [128, 512], F16) for c in range(NC)]
        kTc = [nc.alloc_sbuf_tensor(f"kT{c}", [128, 512], F16) for c in range(NC)]
        w_aug = nc.alloc_sbuf_tensor("w_aug", [128, NT, 130], F16)
        ET = [nc.alloc_sbuf_tensor(f"et{i}", [128, NT, 512], F16) for i in range(2)]
        wk_sb = nc.alloc_sbuf_tensor("wk_sb", [128, 128], F16)
        wvp_sb = nc.alloc_sbuf_tensor("wvp_sb", [128, 128], F16)
        bk_sb = nc.alloc_sbuf_tensor("bk_sb", [128, 1], F32)
        bpe_sb = nc.alloc_sbuf_tensor("bpe_sb", [128, 128], F32)

        nc.sync.dma_start(wk_sb[:], wk[:])
        nc.sync.dma_start(bk_sb[:], bk[:])
        nc.sync.dma_start(wvp_sb[:], wvp[:])

        with (
            tc.tile_pool(name="outp", bufs=4) as outp,
            tc.tile_pool(name="small", bufs=4) as small,
            tc.tile_pool(name="ps", bufs=2, space="PSUM") as psh,
            tc.tile_pool(name="st4", bufs=1, space="PSUM") as st4,
            tc.tile_pool(name="st2", bufs=1, space="PSUM") as st2,
        ):
            uacc = {}

            def s_group(c, mt, g):
                """scores S.T [m-tiles mt..mt+g, n-chunk c] -> exp -> E.T"""
                pool, tag = (st4, "st4") if g == 4 else (st2, "st2")
                stp = pool.tile([128, g * 512], F32, tag=tag)
                for i in range(g):
                    m = mt + i
                    nc.tensor.matmul(
                        stp[:, i * 512:(i + 1) * 512],
                        qTc[m // 4][:, (m % 4) * 128:(m % 4 + 1) * 128],
                        kTc[c][:],
                    )
                nc.scalar.activation(
                    ET[c % 2][:, mt:mt + g, :],
                    stp[:],
                    mybir.ActivationFunctionType.Exp,
                    scale=EXP_SCALE,
                )

            def u_batch(c, mt, g):
                """U += E.T-tiles[mt..mt+g].T @ [w|1] for output chunk c."""
                if c not in uacc:
                    upa = psh.tile([128, 512], F32, tag="ps")
                    upb = psh.tile([128, 512], F32, tag="ps")
                    uacc[c] = (upa, upb)
                ups = uacc[c]
                buf = ET[c % 2]
                for i in range(g):
                    t = mt + i
                    for j in range(4):
                        up = ups[j // 2]
                        off = 129 * (j % 2)
                        nc.tensor.matmul(
                            up[:, off:off + 129],
                            buf[:, t, j * 128:(j + 1) * 128],
                            w_aug[:, t, :129],
                            start=(t == 0 and j % 2 == 0),
                            stop=(t == NT - 1 and j % 2 == 1),
                        )

            def u_final(c):
                """normalize U by its ones-column, add bias, DMA out."""
                ups = uacc.pop(c)
                for j in range(4):
                    up = ups[j // 2]
                    off = 129 * (j % 2)
                    rec = small.tile([128, 1], F32, tag="rec")
                    nc.vector.reciprocal(rec[:], up[:, off + 128:off + 129])
                    ot = outp.tile([128, 128], F32, tag="ot")
                    nc.vector.scalar_tensor_tensor(
                        ot[:],
                        up[:, off:off + 128],
                        rec[:],
                        bpe_sb[:],
                        mybir.AluOpType.mult,
                        mybir.AluOpType.add,
                    )
                    row = c * 512 + j * 128
                    nc.sync.dma_start(out[row:row + 128, :], ot[:])

            def body(_iv=None):
                # phase 1: stream xT/qT chunks (sync + gpsimd HWDGE queues);
                # kT + w_aug per chunk; S(0) groups ride as qT tiles arrive.
                nc.vector.memset(w_aug[:, :, 128:129], 1.0)
                sg = 0
                mt_done = 0
                for c in range(NC):
                    nc.sync.dma_start(xTc[c][:], xt[:, c * 512:(c + 1) * 512])
                    nc.gpsimd.dma_start(qTc[c][:], qt[:, c * 512:(c + 1) * 512])
                    kt = psh.tile([128, 512], F32, tag="ps")
                    nc.tensor.matmul(kt[:], wk_sb[:], xTc[c][:])
                    nc.vector.tensor_scalar_add(kTc[c][:], kt[:], bk_sb[:])
                    if c == 0:
                        nc.sync.dma_start(bpe_sb[:], bpe[:])
                    while sg < len(S_GROUPS) and mt_done + S_GROUPS[sg] <= (c + 1) * 4:
                        s_group(0, mt_done, S_GROUPS[sg])
                        mt_done += S_GROUPS[sg]
                        sg += 1
                    for i in range(4):
                        t = c * 4 + i
                        wp = psh.tile([128, 512], F32, tag="ps")
                        nc.tensor.matmul(
                            wp[:, :128],
                            xTc[c][:, i * 128:(i + 1) * 128],
                            wvp_sb[:],
                        )
                        nc.vector.tensor_copy(w_aug[:, t, :128], wp[:, :128])

                # main pipeline: iteration c emits S-groups of chunk c+1
                # interleaved with U-batches of chunk c (whose exp output
                # was produced during iteration c-1 / phase 1), keeping the
                # ACT exp stream gapless while the PE alternates S and U.
                for c in range(NC):
                    n_s = len(S_GROUPS) if c + 1 < NC else 0
                    for i in range(len(S_GROUPS) + 1):
                        if i < n_s:
                            s_group(c + 1, S_STARTS[i], S_GROUPS[i])
                        if i >= 1:
                            u_batch(c, S_STARTS[i - 1], S_GROUPS[i - 1])
                    u_final(c)

            if reps == 1:
                body()
            else:
                with tc.For_i(0, reps, 1):
                    body()

    nc.compile()
    return nc


def _prep_weights(Wkv, bkv, Wp, bp):
    Wkv = np.asarray(Wkv, np.float32)
    bkv = np.asarray(bkv, np.float32)
    Wp = np.asarray(Wp, np.float32)
    bp = np.asarray(bp, np.float32)
    wk = np.ascontiguousarray(Wkv[:, :D].astype(np.float16))
    bk = np.ascontiguousarray(bkv[:D]).reshape(D, 1)
    wvp = np.ascontiguousarray((Wkv[:, D:] @ Wp).astype(np.float16))
    bpe_row = bkv[D:] @ Wp + bp
    bpe = np.ascontiguousarray(np.tile(bpe_row[None, :], (D, 1)))
    return wk, bk, wvp, bpe


_NC_CACHE = {}


def kernel(x, q_global, Wkv, bkv, Wp, bp):
    xt = np.asarray(x, np.float32).astype(np.float16).transpose(0, 2, 1)
    qt = np.asarray(q_global, np.float32).astype(np.float16).transpose(0, 2, 1)
    wk, bk, wvp, bpe = _prep_weights(Wkv, bkv, Wp, bp)

    if 1 not in _NC_CACHE:
        _NC_CACHE[1] = build(reps=1)
    nc = _NC_CACHE[1]

    in_maps = [
        {
            "xt": np.ascontiguousarray(xt[b]),
            "qt": np.ascontiguousarray(qt[b]),
            "wk": wk,
            "wvp": wvp,
            "bk": bk,
            "bpe": bpe,
        }
        for b in range(B)
    ]
    res = run_bass_kernel_spmd(nc, in_maps, core_ids=list(range(B)))
    return np.stack([res.results[b]["out"] for b in range(B)], axis=0)


# revision 8
# speedup vs baseline: 1.0088x; 1.0088x over previous
"""Trainium2 Bass kernel for nn_AttnGlobal (B=8, N=4096, DIM=128).

reference:
    kv = x @ Wkv + bkv ; k, v = split(kv)
    q = q_global / sqrt(d)
    scores = einsum("bnd,bmd->bnm", k, q)       # softmax over m
    attn = softmax(scores, axis=-1)
    out = einsum("bnm,bmd->bnd", attn, v) @ Wp + bp

Sharding: pure data-parallel over B across the 8 cores (one batch each).

Host-side algebra folds:
    w   = x @ (Wv @ Wp)            (since attn @ (x@Wv) @ Wp = attn @ (x@(Wv@Wp)))
    bpe = bv @ Wp + bp             (since rows of attn sum to 1)

Per-core dataflow:
    xT, qT  : host-pretransposed fp16 inputs        [d, n] / [d, m]
    kT      = Wk.T @ xT + bk                        [d, n]   fp16
    S.T     = qT.T-chunks @ kT                      [m, n] tiles in PSUM (fp32)
    E.T     = exp(S.T / sqrt(d))                    fp16, ACT straight from PSUM
    U_aug   = E @ [w | 1]                           [n, 129] accumulated in PSUM
    out     = U[:, :128] * (1 / U[:, 128]) + bpe    DVE, then DMA out

Schedule: the ACT engine's exp stream (16.8M elems/core @ 1 elem/cyc/lane)
is the hard floor (~133us). The software pipeline keeps ACT gapless:
per chunk c we emit S-groups of chunk c+1 interleaved with U-batches of
chunk c (which consume exp output written one chunk-iteration earlier),
so the PE fills its ACT-slot-wait gaps with U matmuls instead of idling.
DMA triggers live on sync/gpsimd queues (never the scalar engine).
"""

import os
import sys

try:
    import concourse  # noqa: F401  (resolvable via PYTHONPATH on axon images)
except ImportError:
    for _p in ("/opt/trn_rl_repo", os.path.expanduser("~/.axon_site/_ro/trn_rl_repo")):
        if os.path.isdir(_p) and _p not in sys.path:
            sys.path.append(_p)

import numpy as np

import concourse.bacc as bacc
import concourse.mybir as mybir
from concourse.bass_utils import run_bass_kernel_spmd
from concourse.tile import TileContext

B, N, D = 8, 4096, 128
NT = N // 128          # 32 row tiles
NC = N // 512          # 8 column chunks
F32 = mybir.dt.float32
F16 = mybir.dt.float16
EXP_SCALE = 1.0 / float(np.sqrt(D))

# alternating PSUM score-group sizes; sum == NT, st4 uses 4 banks, st2 uses 2
S_GROUPS = [2, 4, 2, 4, 2, 4, 2, 4, 2, 4, 2]
assert sum(S_GROUPS) == NT
S_STARTS = [sum(S_GROUPS[:i]) for i in range(len(S_GROUPS))]


def build(reps: int = 1):
    """Build and compile the per-core Bass program (identical on all cores)."""
    nc = bacc.Bacc("TRN2", target_bir_lowering=False)

    xt = nc.dram_tensor("xt", [D, N], F16, kind="ExternalInput")
    qt = nc.dram_tensor("qt", [D, N], F16, kind="ExternalInput")
    wk = nc.dram_tensor("wk", [D, D], F16, kind="ExternalInput")
    wvp = nc.dram_tensor("wvp", [D, D], F16, kind="ExternalInput")
    bk = nc.dram_tensor("bk", [D, 1], F32, kind="ExternalInput")
    bpe = nc.dram_tensor("bpe", [D, D], F32, kind="ExternalInput")  # row-tiled bias
    out = nc.dram_tensor("out", [N, D], F32, kind="ExternalOutput")

    with TileContext(nc) as tc:
        xTc = [nc.alloc_sbuf_tensor(f"xT{c}", [128, 512], F16) for c in range(NC)]
        qTc = [nc.alloc_sbuf_tensor(f"qT{c}", [128, 512], F16) for c in range(NC)]
        kTc = [nc.alloc_sbuf_tensor(f"kT{c}", [128, 512], F16) for c in range(NC)]
        w_aug = nc.alloc_sbuf_tensor("w_aug", [128, NT, 130], F16)
        ET = [nc.alloc_sbuf_tensor(f"et{i}", [128, NT, 512], F16) for i in range(2)]
        warm_sb = nc.alloc_sbuf_tensor("warm_sb", [128, 128], F16)
        wk_sb = nc.alloc_sbuf_tensor("wk_sb", [128, 128], F16)
        wvp_sb = nc.alloc_sbuf_tensor("wvp_sb", [128, 128], F16)
        bk_sb = nc.alloc_sbuf_tensor("bk_sb", [128, 1], F32)
        bpe_sb = nc.alloc_sbuf_tensor("bpe_sb", [128, 128], F32)

        # weights ride the (idle-at-startup) scalar queue so xt/qt triggers
        # go out first on sync/gpsimd
        nc.scalar.dma_start(wk_sb[:], wk[:])
        nc.scalar.dma_start(bk_sb[:], bk[:])
        nc.scalar.dma_start(wvp_sb[:], wvp[:])
        nc.scalar.dma_start(bpe_sb[:], bpe[:])

        with (
            tc.tile_pool(name="outp", bufs=4) as outp,
            tc.tile_pool(name="small", bufs=4) as small,
            tc.tile_pool(name="ps", bufs=2, space="PSUM") as psh,
            tc.tile_pool(name="st4", bufs=1, space="PSUM") as st4,
            tc.tile_pool(name="st2", bufs=1, space="PSUM") as st2,
        ):
            uacc = {}

            def s_group(c, mt, g, pool=None, tag=None):
                """scores S.T [m-tiles mt..mt+g, n-chunk c] -> exp -> E.T"""
                if pool is None:
                    pool, tag = (st4, "st4") if g == 4 else (st2, "st2")
                stp = pool.tile([128, g * 512], F32, tag=tag)
                for i in range(g):
                    m = mt + i
                    nc.tensor.matmul(
                        stp[:, i * 512:(i + 1) * 512],
                        qTc[m // 4][:, (m % 4) * 128:(m % 4 + 1) * 128],
                        kTc[c][:],
                    )
                nc.scalar.activation(
                    ET[c % 2][:, mt:mt + g, :],
                    stp[:],
                    mybir.ActivationFunctionType.Exp,
                    scale=EXP_SCALE,
                )

            def u_batch(c, mt, g):
                """U += E.T-tiles[mt..mt+g].T @ [w|1] for output chunk c."""
                if c not in uacc:
                    upa = psh.tile([128, 512], F32, tag="ps")
                    upb = psh.tile([128, 512], F32, tag="ps")
                    uacc[c] = (upa, upb)
                ups = uacc[c]
                buf = ET[c % 2]
                for i in range(g):
                    t = mt + i
                    for j in range(4):
                        up = ups[j // 2]
                        off = 129 * (j % 2)
                        nc.tensor.matmul(
                            up[:, off:off + 129],
                            buf[:, t, j * 128:(j + 1) * 128],
                            w_aug[:, t, :129],
                            start=(t == 0 and j % 2 == 0),
                            stop=(t == NT - 1 and j % 2 == 1),
                        )

            def u_final(c):
                """normalize U by its ones-column, add bias, DMA out."""
                ups = uacc.pop(c)
                for j in range(4):
                    up = ups[j // 2]
                    off = 129 * (j % 2)
                    rec = small.tile([128, 1], F32, tag="rec")
                    nc.vector.reciprocal(rec[:], up[:, off + 128:off + 129])
                    ot = outp.tile([128, 128], F32, tag="ot")
                    nc.vector.scalar_tensor_tensor(
                        ot[:],
                        up[:, off:off + 128],
                        rec[:],
                        bpe_sb[:],
                        mybir.AluOpType.mult,
                        mybir.AluOpType.add,
                    )
                    row = c * 512 + j * 128
                    nc.sync.dma_start(out[row:row + 128, :], ot[:])

            def body(_iv=None):
                # HAM warmup: data-independent matmuls into a scratch PSUM
                # slot keep the PE busy while the first input DMAs land, so
                # the 4096-cycle activity window un-throttles the clock gate
                # (K=4/8 -> 8/8) before the real S-matmuls start.
                nc.vector.memset(warm_sb[:], 0.0)
                warm = psh.tile([128, 512], F32, tag="ps")
                for _ in range(6):
                    nc.tensor.matmul(warm[:, :128], warm_sb[:], warm_sb[:])

                # phase 1: stream xT/qT chunks (sync + gpsimd HWDGE queues);
                # kT + w_aug per chunk; S(0) groups ride as qT tiles arrive.
                nc.vector.memset(w_aug[:, :, 128:129], 1.0)
                sg = 0
                mt_done = 0
                for c in range(NC):
                    nc.sync.dma_start(xTc[c][:], xt[:, c * 512:(c + 1) * 512])
                    nc.gpsimd.dma_start(qTc[c][:], qt[:, c * 512:(c + 1) * 512])
                    kt = psh.tile([128, 512], F32, tag="ps")
                    nc.tensor.matmul(kt[:], wk_sb[:], xTc[c][:])
                    nc.vector.tensor_scalar_add(kTc[c][:], kt[:], bk_sb[:])
                    while sg < len(S_GROUPS) and mt_done + S_GROUPS[sg] <= (c + 1) * 4:
                        s_group(0, mt_done, S_GROUPS[sg])
                        mt_done += S_GROUPS[sg]
                        sg += 1
                    for i in range(4):
                        t = c * 4 + i
                        wp = psh.tile([128, 512], F32, tag="ps")
                        nc.tensor.matmul(
                            wp[:, :128],
                            xTc[c][:, i * 128:(i + 1) * 128],
                            wvp_sb[:],
                        )
                        nc.vector.tensor_copy(w_aug[:, t, :128], wp[:, :128])

                # main pipeline: iteration c emits S-groups of chunk c+1
                # interleaved with U-batches of chunk c (whose exp output
                # was produced during iteration c-1 / phase 1), keeping the
                # ACT exp stream gapless while the PE alternates S and U.
                for c in range(NC - 2):
                    for i in range(len(S_GROUPS) + 1):
                        if i < len(S_GROUPS):
                            s_group(c + 1, S_STARTS[i], S_GROUPS[i])
                        if i >= 1:
                            u_batch(c, S_STARTS[i - 1], S_GROUPS[i - 1])
                    u_final(c)

                # last-chunk endgame: the final chunk's S-matmuls serialize
                # with its own exps through the depth-1 PSUM slot rotation
                # (no following chunk hides the turnaround), so use 16
                # groups of 2 ping-ponging across both pools -- the PE can
                # run 2 small groups ahead and the ACT stream stays fed.
                c = NC - 2
                nb = 0
                for i in range(16):
                    pool, tag = (st4, "st4") if i % 2 == 0 else (st2, "st2")
                    s_group(NC - 1, i * 2, 2, pool=pool, tag=tag)
                    while nb < len(S_GROUPS) and nb * 16 <= i * len(S_GROUPS):
                        u_batch(c, S_STARTS[nb], S_GROUPS[nb])
                        nb += 1
                while nb < len(S_GROUPS):
                    u_batch(c, S_STARTS[nb], S_GROUPS[nb])
                    nb += 1
                u_final(c)

                # drain: U of the final chunk trails its exps group-by-group
                c = NC - 1
                for i in range(16):
                    u_batch(c, i * 2, 2)
                u_final(c)

            if reps == 1:
                body()
            else:
                with tc.For_i(0, reps, 1):
                    body()

    nc.compile()
    return nc


def _prep_weights(Wkv, bkv, Wp, bp):
    Wkv = np.asarray(Wkv, np.float32)
    bkv = np.asarray(bkv, np.float32)
    Wp = np.asarray(Wp, np.float32)
    bp = np.asarray(bp, np.float32)
    wk = np.ascontiguousarray(Wkv[:, :D].astype(np.float16))
    bk = np.ascontiguousarray(bkv[:D]).reshape(D, 1)
    wvp = np.ascontiguousarray((Wkv[:, D:] @ Wp).astype(np.float16))
    bpe_row = bkv[D:] @ Wp + bp
    bpe = np.ascontiguousarray(np.tile(bpe_row[None, :], (D, 1)))
    return wk, bk, wvp, bpe


_NC_CACHE = {}


def kernel(x, q_global, Wkv, bkv, Wp, bp):
    xt = np.asarray(x, np.float32).astype(np.float16).transpose(0, 2, 1)
    qt = np.asarray(q_global, np.float32).astype(np.float16).transpose(0, 2, 1)
    wk, bk, wvp, bpe = _prep_weights(Wkv, bkv, Wp, bp)

    if 1 not in _NC_CACHE:
        _NC_CACHE[1] = build(reps=1)
    nc = _NC_CACHE[1]

    in_maps = [
        {
            "xt": np.ascontiguousarray(xt[b]),
            "qt": np.ascontiguousarray(qt[b]),
            "wk": wk,
            "wvp": wvp,
            "bk": bk,
            "bpe": bpe,
        }
        for b in range(B)
    ]
    res = run_bass_kernel_spmd(nc, in_maps, core_ids=list(range(B)))
    return np.stack([res.results[b]["out"] for b in range(B)], axis=0)
